# revision 19
# baseline (speedup 1.0000x reference)
"""Multi-head causal self-attention (B=4, N=2048, D=1024, H=16) on 8 TRN2 cores.

Sharding: 8 cores = 4 batches x 2 head-groups (8 heads / 512 dims each).
Per core (batch b, group g):
  - QKV projections computed in transposed layout (dims on partitions):
      Q^T, K^T = W^T-chunks (lhsT) x x^T (rhs), accumulated over 8 din chunks.
      V computed in natural [token, dv] layout (lhsT = x^T chunk).
  - Attention computed as S^T tiles [keys(128) x queries(512)] so that
    exp(S) feeds the P^T.V matmul directly (contraction over keys on
    partitions, no transposes anywhere). The two heads of a 128-partition
    strip are processed as a PAIR: their S^T matmuls (K=hd=64) go to PE
    row-groups 0/64 via base partitions, so adjacent instructions run
    CONCURRENTLY on the two array halves (~2x on the S phase). Softmax
    denominators come from a ones-column appended to V (row HD of the PV
    accumulator); normalization is per-query-strip, deferred into later
    units via a background work queue. Causal masking = skip blocks above
    the diagonal + multiply diagonal-region tiles by a precomputed 0/1
    mask after exp. No max-subtraction: scores are ~N(0,1) after the
    1/sqrt(hd) scale, exp is safe in fp32.
  - The kernel is software-pipelined with a background queue: Q/K
    projections for strip s+1, the V projection (strip 0), softmax
    normalizations, and O-projection token groups (strip 3) are woven
    between attention units, filling the PE while the Scalar engine
    (exp, the co-critical resource at ~1 elem/cycle/lane) catches up.
  - O-projection: attnT (lhsT) x Wo-slice (rhs) -> [2048, 1024] partial
    output per core; host sums the two group partials per batch. A token
    group (tt, half) is emitted as soon as strip 3's rows for its query
    strip are normalized.

Dtypes: bf16 matmuls everywhere (fp32 PSUM); probs bf16 in [0,1].
"""

import numpy as np
import ml_dtypes

import concourse.bass as bass
import concourse.tile as tile
from concourse import bacc, mybir
from concourse import bass_utils
from concourse._compat import with_exitstack
from concourse.bass import ts, ds

B, N, D, H, HD = 4, 2048, 1024, 16, 64
GROUPS = 2              # head groups (cores per batch)
DC = D // GROUPS        # 512 dims per core
HPC = H // GROUPS       # 8 heads per core
P = 128
QW = 512                # query strip width / matmul free dim
NDIN = D // P           # 8 contraction chunks for QKV
NSTRIP = DC // P        # 4 dq strips per core (2 heads each)
NTT = N // P            # 16 token tiles
NTS = N // QW           # 4 token strips
NQB = QW // P           # 4 query blocks per strip

F32 = mybir.dt.float32
BF16 = mybir.dt.bfloat16


def _emit(ctx, tc, xT, wq, wk, wv, wo, bq, bk, bv, masks, out, dbg=None):
    nc = tc.nc
    EXP = mybir.ActivationFunctionType.Exp

    const = ctx.enter_context(tc.tile_pool(name="const", bufs=1))
    p_pt = ctx.enter_context(tc.tile_pool(name="p_pt", bufs=3))
    p_small = ctx.enter_context(tc.tile_pool(name="p_small", bufs=2))
    p_dram = ctx.enter_context(tc.tile_pool(name="p_dram", bufs=3, space="DRAM"))
    p_osb = ctx.enter_context(tc.tile_pool(name="p_osb", bufs=3))
    # PSUM: exactly 8 banks. st0/st1 = single-buffered per-head S^T tiles
    # (2 banks each), pv0/pv1 = PV accumulators (1 bank each), mm = shared
    # by QKV projection rounds and O-projection groups (2 banks).
    p_mm = ctx.enter_context(tc.tile_pool(name="p_mm", bufs=2, space="PSUM"))
    p_st = ctx.enter_context(tc.tile_pool(name="p_st", bufs=2, space="PSUM"))
    p_pv = ctx.enter_context(tc.tile_pool(name="p_pv", bufs=2, space="PSUM"))
    p_w = ctx.enter_context(tc.tile_pool(name="p_w", bufs=2))
    p_qk = ctx.enter_context(tc.tile_pool(name="p_qk", bufs=3))
    p_xt = ctx.enter_context(tc.tile_pool(name="p_xt", bufs=1))

    wqr = wq.rearrange("(c p) f -> p c f", p=P)
    wkr = wk.rearrange("(c p) f -> p c f", p=P)
    wvr = wv.rearrange("(c p) f -> p c f", p=P)
    wor = wo.rearrange("(c p) f -> p c f", p=P)
    xTr = xT.rearrange("(c p) n -> c p n", p=P)

    # ---- weight/const loads; strip-0 Q/K weights first (critical path) ----
    wqks = []
    wqks.append((p_w.tile([P, NDIN, P], BF16, tag="wq", name="wq0"),
                 p_w.tile([P, NDIN, P], BF16, tag="wk", name="wk0")))
    nc.gpsimd.dma_start(out=wqks[0][0], in_=wqr[:, :, ts(0, P)])
    nc.gpsimd.dma_start(out=wqks[0][1], in_=wkr[:, :, ts(0, P)])

    # x^T resident, 8 chunk DMAs of [128, 2048] on the sync queue
    xt = p_xt.tile([P, NDIN, N], BF16)
    for c in range(NDIN):
        nc.sync.dma_start(out=xt[:, c, :], in_=xTr[c])

    wvt = const.tile([P, NDIN, DC], BF16)
    nc.gpsimd.dma_start(out=wvt, in_=wvr)
    wot = const.tile([P, NSTRIP, D], BF16)
    nc.gpsimd.dma_start(out=wot, in_=wor)

    maskt = const.tile([P, P], BF16)
    nc.gpsimd.dma_start(out=maskt, in_=masks)
    bqt = const.tile([P, NSTRIP], F32)
    nc.gpsimd.dma_start(out=bqt, in_=bq.rearrange("(s p) -> p s", p=P))
    bkt = const.tile([P, NSTRIP], F32)
    nc.gpsimd.dma_start(out=bkt, in_=bk.rearrange("(s p) -> p s", p=P))
    bvb = const.tile([P, DC], F32)
    nc.gpsimd.dma_start(out=bvb, in_=bv.unsqueeze(0).partition_broadcast(P))

    # persistent per-batch tensors
    attnT = const.tile([P, NSTRIP, N], BF16)                # normalized attn^T
    vplus = const.tile([P, NTT, HPC, HD + 1], BF16)         # V | ones column
    # memset on an f32r/bf16 matmul-input tile is invalid ISA; write the ones
    # column via a DVE copy from an f32 staging tile (a valid rounding producer)
    ones_f32 = const.tile([P, NTT * HPC], F32)
    nc.vector.memset(ones_f32, 1.0)
    nc.vector.tensor_copy(
        out=vplus[:, :, :, HD:HD + 1],
        in_=ones_f32.rearrange("p (a b) -> p a b", b=HPC).unsqueeze(3),
    )
    ones_row = const.tile([1, P], F32)
    nc.vector.memset(ones_row, 1.0)
    dbg_sums = const.tile([P, NSTRIP, 2, QW], F32)
    dbg_pre = const.tile([P, NSTRIP, N], BF16)

    # ---- background work queue -------------------------------------------
    # Ordered (pe_cost_us, key, closure) items; consumed between attention
    # units to fill the PE while the Scalar engine works through the exps.
    # `need(key)` force-runs from the front until `key` has executed, so
    # producers can be woven lazily yet land before their first consumer.
    bg = []
    bg_queued = set()
    bg_done = set()

    def bg_put(cost, key, fn):
        bg.append((cost, key, fn))
        bg_queued.add(key)

    def _run_one():
        cost, key, fn = bg.pop(0)
        fn()
        bg_done.add(key)
        return cost

    def run_bg(budget):
        spent = 0.0
        while bg and spent < budget:
            spent += _run_one()

    def need(key):
        if key not in bg_queued:
            return
        while key not in bg_done:
            _run_one()

    def drain_bg():
        while bg:
            _run_one()

    # ---- emitters ---------------------------------------------------------
    def v_round(tt):
        psv = p_mm.tile([P, DC], F32, tag="mm", name="psv")
        for c in range(NDIN):
            nc.tensor.matmul(
                psv, lhsT=xt[:, c, ts(tt, P)], rhs=wvt[:, c, :],
                start=(c == 0), stop=(c == NDIN - 1),
            )
        nc.vector.tensor_add(
            out=vplus[:, tt, :, 0:HD],
            in0=psv.rearrange("p (h d) -> p h d", d=HD),
            in1=bvb.rearrange("p (h d) -> p h d", d=HD),
        )

    def need_v(tt_max):
        need(("v", tt_max))

    def proj_round(wqs_or_wks, bias, t, qkts, s):
        ps = p_mm.tile([P, QW], F32, tag="mm", name="psqk")
        for c in range(NDIN):
            nc.tensor.matmul(
                ps, lhsT=wqs_or_wks[:, c, :], rhs=xt[:, c, ts(t, QW)],
                start=(c == 0), stop=(c == NDIN - 1),
            )
        nc.vector.tensor_scalar_add(
            out=qkts[:, ts(t, QW)], in0=ps, scalar1=bias[:, s:s + 1])

    def o_group(tt, half):
        pso = p_mm.tile([P, QW], F32, tag="mm", name="pso")
        for c in range(NSTRIP):
            nc.tensor.matmul(
                pso, lhsT=attnT[:, c, ts(tt, P)],
                rhs=wot[:, c, ds(half * QW, QW)],
                start=(c == 0), stop=(c == NSTRIP - 1),
            )
        osb = p_osb.tile([P, QW], F32, tag="osb", name="osb")
        nc.vector.tensor_copy(out=osb, in_=pso)
        nc.sync.dma_start(out=out[ts(tt, P), ds(half * QW, QW)], in_=osb)

    def normalize_h2(s, h2, sums_sb):
        """Batched softmax normalization for one head (4 query strips),
        rows at partition offsets {0,32,64,96} (the step-1-validated path)."""
        po = h2 * HD
        recip_sb = p_small.tile([P, QW], F32, tag="recip", name="recip_sb")
        nc.vector.reciprocal_approx_fast(out=recip_sb, in_=sums_sb[:, h2, :])
        recip_d = p_dram.tile([NTS, QW], F32, tag="rd", name="recip_d")
        nc.sync.dma_start(
            out=recip_d,
            in_=recip_sb.rearrange("(a b) f -> a b f", b=32)[:, 0, :])
        for qs in range(NTS):
            rb = p_small.tile([P, QW], F32, tag="rb", bufs=3, name="rb")
            nc.sync.dma_start(
                out=rb, in_=recip_d[qs, :].unsqueeze(0).partition_broadcast(P))
            sl = attnT[po:po + HD, s, ts(qs, QW)]
            nc.vector.tensor_mul(out=sl, in0=sl, in1=rb[po:po + HD, :])

    def attn_head(s, qs, h2, qts, kts, sums_sb, budget):
        """S^T/exp/PV for one (head, query strip), original unit layout.

        Work units: full-width kc pairs below the diagonal region, then two
        packed diagonal units with shrinking query widths (512+384 and
        256+128).
        """
        po = h2 * HD
        h = 2 * s + h2
        nfull = NQB * qs             # unmasked key blocks 0..nfull-1
        q0 = qs * QW
        pvp = p_pv.tile([HD + 1, QW], F32, tag="pv", name="pvp")

        units = [("full", ip) for ip in range(nfull // 2)]
        units.append(("diagA", None))
        units.append(("diagB", None))

        def emit_s(unit):
            kind, ip = unit
            pst = p_st.tile([P, 2, QW], F32, tag="st", name="pst")
            pt = p_pt.tile([P, 2, QW], BF16, tag="pt", name="pt")
            if kind == "full":
                for j2 in range(2):
                    kc = 2 * ip + j2
                    nc.tensor.matmul(
                        pst[:, j2, :],
                        lhsT=kts[po:po + HD, ts(kc, P)],
                        rhs=qts[po:po + HD, ts(qs, QW)],
                        start=True, stop=True,
                    )
                nc.scalar.activation(out=pt, in_=pst, func=EXP, scale=0.125)
            elif kind == "diagA":
                # j=0: kc=nfull,   queries [0:512), tri on cols 0:128
                # j=1: kc=nfull+1, queries [128:512), tri on cols 0:128
                nc.tensor.matmul(
                    pst[:, 0, :],
                    lhsT=kts[po:po + HD, ts(nfull, P)],
                    rhs=qts[po:po + HD, ts(qs, QW)],
                    start=True, stop=True,
                )
                nc.tensor.matmul(
                    pst[:, 1, 0:3 * P],
                    lhsT=kts[po:po + HD, ts(nfull + 1, P)],
                    rhs=qts[po:po + HD, ds(q0 + P, 3 * P)],
                    start=True, stop=True,
                )
                nc.scalar.activation(
                    out=pt[:, 0, :], in_=pst[:, 0, :], func=EXP, scale=0.125)
                nc.scalar.activation(
                    out=pt[:, 1, 0:3 * P], in_=pst[:, 1, 0:3 * P],
                    func=EXP, scale=0.125)
                nc.vector.tensor_mul(pt[:, 0, 0:P], pt[:, 0, 0:P], maskt)
                nc.vector.tensor_mul(pt[:, 1, 0:P], pt[:, 1, 0:P], maskt)
            else:
                # diagB: j=2: kc=nfull+2, queries [256:512) at cols 0:256;
                #        j=3: kc=nfull+3, queries [384:512) at cols 256:384
                nc.tensor.matmul(
                    pst[:, 0, 0:2 * P],
                    lhsT=kts[po:po + HD, ts(nfull + 2, P)],
                    rhs=qts[po:po + HD, ds(q0 + 2 * P, 2 * P)],
                    start=True, stop=True,
                )
                nc.tensor.matmul(
                    pst[:, 0, 2 * P:3 * P],
                    lhsT=kts[po:po + HD, ts(nfull + 3, P)],
                    rhs=qts[po:po + HD, ds(q0 + 3 * P, P)],
                    start=True, stop=True,
                )
                nc.scalar.activation(
                    out=pt[:, 0, 0:3 * P], in_=pst[:, 0, 0:3 * P],
                    func=EXP, scale=0.125)
                nc.vector.tensor_mul(pt[:, 0, 0:P], pt[:, 0, 0:P], maskt)
                nc.vector.tensor_mul(
                    pt[:, 0, 2 * P:3 * P], pt[:, 0, 2 * P:3 * P], maskt)
            return pt

        def emit_pv(unit, pt):
            kind, ip = unit
            if kind == "full":
                if s == 0:
                    need_v(2 * ip + 1)
                for j2 in range(2):
                    kc = 2 * ip + j2
                    nc.tensor.matmul(
                        pvp, lhsT=vplus[:, kc, h, :], rhs=pt[:, j2, :],
                        start=(kc == 0), stop=False,
                    )
            elif kind == "diagA":
                if s == 0:
                    need_v(nfull + 1)
                nc.tensor.matmul(
                    pvp, lhsT=vplus[:, nfull, h, :], rhs=pt[:, 0, :],
                    start=(nfull == 0), stop=False,
                )
                nc.tensor.matmul(
                    pvp[:, P:4 * P], lhsT=vplus[:, nfull + 1, h, :],
                    rhs=pt[:, 1, 0:3 * P], start=False, stop=False,
                )
            else:
                if s == 0:
                    need_v(nfull + 3)
                nc.tensor.matmul(
                    pvp[:, 2 * P:4 * P], lhsT=vplus[:, nfull + 2, h, :],
                    rhs=pt[:, 0, 0:2 * P], start=False, stop=False,
                )
                nc.tensor.matmul(
                    pvp[:, 3 * P:4 * P], lhsT=vplus[:, nfull + 3, h, :],
                    rhs=pt[:, 0, 2 * P:3 * P], start=False, stop=True,
                )

        pend = {}
        for i in range(len(units) + 1):
            if i < len(units):
                run_bg(budget)
                pend[i] = emit_s(units[i])
            if i >= 1:
                emit_pv(units[i - 1], pend.pop(i - 1))
        r = 32 * qs
        nc.vector.tensor_copy(
            out=sums_sb[r:r + 1, h2, :], in_=pvp[HD:HD + 1, :])
        nc.vector.tensor_copy(
            out=attnT[po:po + HD, s, ts(qs, QW)], in_=pvp[0:HD, :])

    def attn_qs(s, qs, qts, kts, sums_sb, budget):
        attn_head(s, qs, 0, qts, kts, sums_sb, budget)
        attn_head(s, qs, 1, qts, kts, sums_sb, budget)

    # ---- main schedule ----------------------------------------------------
    # V rounds queued first (needed by strip-0 attention, tt-ordered).
    for tt in range(NTT):
        bg_put(1.8, ("v", tt), lambda t=tt: v_round(t))

    qkts = {0: (p_qk.tile([P, N], BF16, tag="qt", name="qts"),
                p_qk.tile([P, N], BF16, tag="kt", name="kts"))}
    for s in range(NSTRIP):
        # prefetch next strip's weights; queue its projection rounds
        if s + 1 < NSTRIP:
            wq_n = p_w.tile([P, NDIN, P], BF16, tag="wq", name="wqn")
            wk_n = p_w.tile([P, NDIN, P], BF16, tag="wk", name="wkn")
            nc.gpsimd.dma_start(out=wq_n, in_=wqr[:, :, ts(s + 1, P)])
            nc.gpsimd.dma_start(out=wk_n, in_=wkr[:, :, ts(s + 1, P)])
            wqks.append((wq_n, wk_n))
            qts_n = p_qk.tile([P, N], BF16, tag="qt", name="qts")
            kts_n = p_qk.tile([P, N], BF16, tag="kt", name="kts")
            qkts[s + 1] = (qts_n, kts_n)
            for t in range(NTS):
                bg_put(1.8, ("proj", s + 1, "q", t),
                       lambda t=t, s1=s + 1, w=wq_n, q=qts_n:
                       proj_round(w, bqt, t, q, s1))
                bg_put(1.8, ("proj", s + 1, "k", t),
                       lambda t=t, s1=s + 1, w=wk_n, k=kts_n:
                       proj_round(w, bkt, t, k, s1))

        qts, kts = qkts.pop(s)
        sums_sb = p_small.tile([P, 2, QW], F32, tag="sums", name="sums")
        nc.gpsimd.memset(sums_sb, 1.0)

        for t in range(NTS):
            if s == 0:
                # strip-0 projections are the critical path: emit inline
                proj_round(wqks[0][0], bqt, t, qts, 0)
                proj_round(wqks[0][1], bkt, t, kts, 0)
            else:
                # keys up to block 4t+3 and queries strip t must exist
                need(("proj", s, "q", t))
                need(("proj", s, "k", t))
            attn_qs(s, t, qts, kts, sums_sb, budget=1.4)
        if s == NSTRIP - 1:
            normalize_h2(s, 0, sums_sb)
            normalize_h2(s, 1, sums_sb)
            for tt in range(NTT):
                for half in range(2):
                    bg_put(1.1, ("o", tt, half),
                           lambda a=tt, b=half: o_group(a, b))
        else:
            bg_put(0.2, ("norm", s, 0),
                   lambda ss=s, sb=sums_sb: normalize_h2(ss, 0, sb))
            bg_put(0.2, ("norm", s, 1),
                   lambda ss=s, sb=sums_sb: normalize_h2(ss, 1, sb))

    drain_bg()
    if dbg is not None:
        nc.sync.dma_start(out=dbg["attn"], in_=attnT)
        nc.sync.dma_start(out=dbg["vplus"], in_=vplus)
        nc.sync.dma_start(out=dbg["sums"], in_=dbg_sums)
        nc.sync.dma_start(out=dbg["pre"], in_=dbg_pre)


_emit_wrapped = with_exitstack(_emit)

_NC_CACHE = None


def _build():
    global _NC_CACHE
    if _NC_CACHE is not None:
        return _NC_CACHE
    nc = bacc.Bacc("TRN2", target_bir_lowering=False, debug=False)
    xT = nc.dram_tensor("xt", [D, N], BF16, kind="ExternalInput").ap()
    wq = nc.dram_tensor("wq", [D, DC], BF16, kind="ExternalInput").ap()
    wk = nc.dram_tensor("wk", [D, DC], BF16, kind="ExternalInput").ap()
    wv = nc.dram_tensor("wv", [D, DC], BF16, kind="ExternalInput").ap()
    wo = nc.dram_tensor("wo", [DC, D], BF16, kind="ExternalInput").ap()
    bq = nc.dram_tensor("bq", [DC], F32, kind="ExternalInput").ap()
    bk = nc.dram_tensor("bk", [DC], F32, kind="ExternalInput").ap()
    bv = nc.dram_tensor("bv", [DC], F32, kind="ExternalInput").ap()
    masks = nc.dram_tensor("masks", [P, P], BF16, kind="ExternalInput").ap()
    out = nc.dram_tensor("out", [N, D], F32, kind="ExternalOutput").ap()
    dbg = None
    import os
    if os.environ.get("KDBG"):
        dbg = {
            "attn": nc.dram_tensor("dbg_attn", [P, NSTRIP, N], BF16,
                                   kind="ExternalOutput").ap(),
            "vplus": nc.dram_tensor("dbg_vplus", [P, NTT, HPC, HD + 1], BF16,
                                    kind="ExternalOutput").ap(),
            "sums": nc.dram_tensor("dbg_sums", [P, NSTRIP, 2, QW], F32,
                                   kind="ExternalOutput").ap(),
            "pre": nc.dram_tensor("dbg_pre", [P, NSTRIP, N], BF16,
                                  kind="ExternalOutput").ap(),
        }
    with tile.TileContext(nc) as tc:
        _emit_wrapped(tc, xT, wq, wk, wv, wo, bq, bk, bv, masks, out, dbg)
    nc.compile()
    _NC_CACHE = nc
    return nc


def _make_masks():
    # triangular 0/1 tile for the diagonal blocks of S^T: key <= query kept
    return np.triu(np.ones((P, P), np.float32)).astype(ml_dtypes.bfloat16)


def _in_maps(x, Wq, bq, Wk, bk, Wv, bv, Wo):
    masks = _make_masks()
    maps = []
    for b in range(B):
        xt_b = np.ascontiguousarray(np.asarray(x[b]).T)
        for g in range(GROUPS):
            sl = slice(g * DC, (g + 1) * DC)
            bf = ml_dtypes.bfloat16
            maps.append({
                "xt": xt_b.astype(bf),
                "wq": np.ascontiguousarray(Wq[:, sl]).astype(bf),
                "wk": np.ascontiguousarray(Wk[:, sl]).astype(bf),
                "wv": np.ascontiguousarray(Wv[:, sl]).astype(bf),
                "wo": np.ascontiguousarray(Wo[sl, :]).astype(bf),
                "bq": np.ascontiguousarray(bq[sl]),
                "bk": np.ascontiguousarray(bk[sl]),
                "bv": np.ascontiguousarray(bv[sl]),
                "masks": masks,
            })
    return maps


def run(inputs, trace=False, tmpdir=None):
    """Build+run on 8 cores. Returns (out [B,N,D] f32, BassKernelResults)."""
    x = np.asarray(inputs["x"], np.float32)
    args = [np.asarray(inputs[k], np.float32) for k in
            ("Wq", "bq", "Wk", "bk", "Wv", "bv", "Wo")]
    bo = np.asarray(inputs["bo"], np.float32)
    nc = _build()
    maps = _in_maps(x, *args)
    if trace:
        bass_utils.upload_artifacts = lambda d: d
    res = bass_utils.run_bass_kernel_spmd(
        nc, maps, core_ids=list(range(8)), trace=trace, tmpdir=tmpdir)
    out = np.empty((B, N, D), np.float32)
    for b in range(B):
        out[b] = res.results[2 * b]["out"] + res.results[2 * b + 1]["out"] + bo
    return out, res


def kernel(**inputs):
    out, _ = run(inputs)
    return out
